# revision 1
# baseline (speedup 1.0000x reference)
"""2-layer GCN (PyG GCNConv semantics) on 8 Trainium2 NeuronCores.

Strategy (dst-sharded message passing):
  - Nodes are split into 8 contiguous blocks of 6250 rows; core c owns output
    rows [6250c, 6250(c+1)).  Edges (plus one self-loop per node) are
    partitioned by destination core, then by 256-node destination windows,
    then packed into 128-edge tiles.
  - Per edge tile: gather x[src] rows from HBM with dma_gather (one
    descriptor per edge; the Q7 descriptor-generation rate of ~6.3 ns/row is
    the kernel's bottleneck), build a norm-scaled one-hot dst selection
    matrix in a single DVE tensor_scalar (2x_2p mode:
    sel = (iota == dst_local) * norm), and accumulate
    aggT[feat, dst] += msg^T @ sel on the PE array in float32r (full-rate
    fp32 at N=256, ~1.6e-4 rounding).  Per-window exact valid-index counts
    are loaded into gpsimd registers (num_idxs_reg) so tile padding costs no
    descriptors.
  - Window flush (layer 1): h1T = relu(W1^T @ aggT + b1); pT = W2^T @ h1T;
    PE-transpose to row-major and store p = h1 @ W2 rows to DRAM.
  - AllGather p across the 8 cores (12.8 MB), then layer 2 re-uses the same
    edge tiles gathering 256B rows from p_full, adds b2, transposes, stores.
  - dma_gather indices are int16, so the 50000-row tables are addressed via
    two calls per window: "lo" (src < 32768, base table) and "hi"
    (src >= 32768, an offset view of the same table).  Padding uses trailing
    -1 indices which the DGE skips (no DMA traffic); padded lanes are killed
    in the selection matrix (dst_local = -1 never matches the iota).

Host-side work is index preprocessing only (degrees/norms from edge_index,
sorting, packing); all FLOPs on the gathered/aggregated features run on
device.
"""

import os
import sys

import numpy as np

for _p in ("/opt/trn_rl_repo", "/root/.axon_site/_ro/trn_rl_repo"):
    if os.path.isdir(_p) and _p not in sys.path:
        sys.path.insert(0, _p)

import concourse.bacc as bacc
import concourse.tile as tile
from concourse import mybir
from concourse.bass_utils import run_bass_kernel_spmd

P = 128
N_NODES = 50000
C_IN = 128
C_HID = 128
C_OUT = 64
CORES = 8
BLOCK = N_NODES // CORES          # 6250
WIN = 256                         # dst nodes per PSUM window
NW = -(-BLOCK // WIN)             # 25 windows per core (last has 106 rows)
SPLIT = 32768                     # int16-addressable base-table rows
GAT_BUFS = 4                      # msg tile double/triple buffering


def _preprocess(edge_index):
    """Partition edges by (core, window, lo/hi), pad tiles, build device arrays."""
    src = np.asarray(edge_index[0], dtype=np.int64)
    dst = np.asarray(edge_index[1], dtype=np.int64)

    deg = np.bincount(dst, minlength=N_NODES).astype(np.float64) + 1.0
    dinv = 1.0 / np.sqrt(deg)

    a_src = np.concatenate([src, np.arange(N_NODES, dtype=np.int64)])
    a_dst = np.concatenate([dst, np.arange(N_NODES, dtype=np.int64)])
    a_nrm = np.concatenate([dinv[src] * dinv[dst], dinv * dinv]).astype(np.float32)

    core = a_dst // BLOCK
    win = (a_dst % BLOCK) // WIN
    dloc = ((a_dst % BLOCK) % WIN).astype(np.float32)
    is_hi = (a_src >= SPLIT).astype(np.int64)

    key = (core * NW + win) * 2 + is_hi
    order = np.argsort(key, kind="stable")
    cnt = np.bincount(key, minlength=CORES * NW * 2)
    cnt3 = cnt.reshape(CORES, NW, 2)
    t_lo = int(-(-cnt3[:, :, 0].max() // P))
    t_hi = int(-(-cnt3[:, :, 1].max() // P))

    s_src = a_src[order]
    s_nrm = a_nrm[order]
    s_dloc = dloc[order]
    bounds = np.concatenate([[0], np.cumsum(cnt)])

    t_w = t_lo + t_hi
    per_core = []
    for c in range(CORES):
        idx_lo = np.full((NW, t_lo * P), -1, np.int16)
        idx_hi = np.full((NW, t_hi * P), -1, np.int16)
        dl = np.full((NW, t_w * P), -1.0, np.float32)
        nm = np.zeros((NW, t_w * P), np.float32)
        for w in range(NW):
            k = (c * NW + w) * 2
            lo0, lo1 = bounds[k], bounds[k + 1]
            hi0, hi1 = bounds[k + 1], bounds[k + 2]
            nlo, nhi = lo1 - lo0, hi1 - hi0
            idx_lo[w, :nlo] = s_src[lo0:lo1].astype(np.int16)
            idx_hi[w, :nhi] = (s_src[hi0:hi1] - SPLIT).astype(np.int16)
            dl[w, :nlo] = s_dloc[lo0:lo1]
            dl[w, t_lo * P:t_lo * P + nhi] = s_dloc[hi0:hi1]
            nm[w, :nlo] = s_nrm[lo0:lo1]
            nm[w, t_lo * P:t_lo * P + nhi] = s_nrm[hi0:hi1]

        # int16 index arrays: idx i of a call lives at [i % 16, i // 16],
        # replicated across the 8 groups of 16 partitions (one per Q7 core).
        def wrap16(a):  # [NW, L] -> [128, NW * L / 16]
            L = a.shape[1]
            w16 = a.reshape(NW, L // 16, 16).transpose(2, 0, 1).reshape(16, -1)
            return np.tile(w16, (8, 1))

        # per-lane tile arrays: edge (w, tile t, lane p) -> column w*t_w + t,
        # partition p.
        def lanes(a):  # [NW, t_w * P] -> [128, NW * t_w]
            return a.reshape(NW, t_w, P).transpose(2, 0, 1).reshape(P, -1)

        cnts = np.zeros((NW, 2), np.int32)
        for w in range(NW):
            k = (c * NW + w) * 2
            cnts[w, 0] = bounds[k + 1] - bounds[k]
            cnts[w, 1] = bounds[k + 2] - bounds[k + 1]
        per_core.append({
            "idx_lo": wrap16(idx_lo),
            "idx_hi": wrap16(idx_hi),
            "dloc": lanes(dl),
            "nrm": lanes(nm),
            "cnt": cnts.reshape(1, -1),
        })
    return t_lo, t_hi, per_core


_BUILD_CACHE = {}

# 0 = full kernel; 1 = layer 1 only (out <- p rows, no collective/layer 2);
# 2 = layers + collective but layer 2 gathers from p_mine (no collective dep)
DEBUG_STAGE = 0


def _build(t_lo, t_hi):
    if (t_lo, t_hi) in _BUILD_CACHE:
        return _BUILD_CACHE[(t_lo, t_hi)]

    t_w = t_lo + t_hi
    t_total = NW * t_w
    # meta f32 columns: [dloc | nrm | iota(WIN) | b1 | b2]
    c_dst, c_nrm, c_iota = 0, t_total, 2 * t_total
    c_b1, c_b2 = 2 * t_total + WIN, 2 * t_total + WIN + 1
    meta_cols = 2 * t_total + WIN + 2

    f32, f32r, i16 = mybir.dt.float32, mybir.dt.float32r, mybir.dt.int16
    RELU = mybir.ActivationFunctionType.Relu
    COPY = mybir.ActivationFunctionType.Copy
    IDENT = mybir.ActivationFunctionType.Identity

    nc = bacc.Bacc("TRN2", num_devices=CORES, num_swdge_queues=4)
    x_ext = nc.dram_tensor("x", [N_NODES, C_IN], f32r, kind="ExternalInput")
    ilo_ext = nc.dram_tensor("idx_lo", [P, NW * t_lo * 8], i16, kind="ExternalInput")
    ihi_ext = nc.dram_tensor("idx_hi", [P, NW * t_hi * 8], i16, kind="ExternalInput")
    meta_ext = nc.dram_tensor("meta", [P, meta_cols], f32, kind="ExternalInput")
    cnt_ext = nc.dram_tensor("cnt", [1, NW * 2], mybir.dt.int32, kind="ExternalInput")
    w_ext = nc.dram_tensor("wts", [P, 256], f32r, kind="ExternalInput")
    out_ext = nc.dram_tensor("out", [BLOCK, C_OUT], f32, kind="ExternalOutput")

    with tile.TileContext(nc) as tc:
        with tc.tile_pool(name="const", bufs=1) as cpool, \
             tc.tile_pool(name="gat", bufs=GAT_BUFS) as gpool, \
             tc.tile_pool(name="work", bufs=3) as wpool, \
             tc.tile_pool(name="flush", bufs=2) as fpool, \
             tc.tile_pool(name="dram", bufs=1, space="DRAM") as dpool, \
             tc.tile_pool(name="ps_agg", bufs=2, space="PSUM") as ps_agg, \
             tc.tile_pool(name="ps_z", bufs=2, space="PSUM") as ps_z, \
             tc.tile_pool(name="ps_pt", bufs=2, space="PSUM") as ps_pt, \
             tc.tile_pool(name="ps_rm", bufs=2, space="PSUM") as ps_rm:

            ilo_s = cpool.tile([P, NW * t_lo * 8], i16)
            ihi_s = cpool.tile([P, NW * t_hi * 8], i16)
            meta_s = cpool.tile([P, meta_cols], f32)
            w_s = cpool.tile([P, 256], f32r)
            cnt_s = cpool.tile([1, NW * 2], mybir.dt.int32)
            nc.sync.dma_start(out=cnt_s[:], in_=cnt_ext[:])
            nc.sync.dma_start(out=ilo_s[:], in_=ilo_ext[:])
            nc.sync.dma_start(out=ihi_s[:], in_=ihi_ext[:])
            nc.sync.dma_start(out=meta_s[:], in_=meta_ext[:])
            nc.sync.dma_start(out=w_s[:], in_=w_ext[:])

            p_mine = dpool.tile([BLOCK, C_OUT], f32r)
            p_full = dpool.tile([CORES * BLOCK, C_OUT], f32r)

            def window_tiles(layer, w, msg, feat):
                """Per-tile: fused norm-scaled one-hot sel (one DVE tensor_scalar,
                2x_2p mode) + PE accumulation with the raw gathered rows."""
                agg = ps_agg.tile([P, WIN], f32, space="PSUM", tag="agg")
                for t in range(t_w):
                    col = w * t_w + t
                    sel = wpool.tile([P, WIN], f32r, tag="sel")
                    nc.vector.tensor_scalar(
                        out=sel[:],
                        in0=meta_s[:, c_iota:c_iota + WIN],
                        scalar1=meta_s[:, c_dst + col:c_dst + col + 1],
                        scalar2=meta_s[:, c_nrm + col:c_nrm + col + 1],
                        op0=mybir.AluOpType.is_equal,
                        op1=mybir.AluOpType.mult,
                    )
                    nc.tensor.matmul(
                        out=agg[:feat, :], lhsT=msg[:, t, :], rhs=sel[:],
                        start=(t == 0), stop=(t == t_w - 1),
                    )
                return agg

            def store_rowmajor(w, colT_s, dram_dst, dt_out):
                """colT_s [64, WIN] -> transpose halves -> rows of dram_dst."""
                rows = min(WIN, BLOCK - w * WIN)
                for h in range((rows + P - 1) // P):
                    rh = min(P, rows - h * P)
                    rm = ps_rm.tile([P, C_OUT], f32r, space="PSUM", tag="rm")
                    nc.tensor.transpose(
                        out=rm[:],
                        in_=colT_s[:, h * P:(h + 1) * P],
                        identity=w_s[0:C_OUT, 192:256],
                    )
                    rm_s = fpool.tile([P, C_OUT], dt_out, tag="rm_s")
                    nc.scalar.activation(out=rm_s[:], in_=rm[:].bitcast(f32), func=COPY)
                    r0 = w * WIN + h * P
                    nc.sync.dma_start(out=dram_dst[r0:r0 + rh, :], in_=rm_s[:rh, :])

            # ---------------- layer 1 ----------------
            import contextlib
            _rctx = contextlib.ExitStack()
            rlo = _rctx.enter_context(nc.gpsimd.register("rlo"))
            rhi = _rctx.enter_context(nc.gpsimd.register("rhi"))
            for w in range(NW):
                msg = gpool.tile([P, t_w, C_IN], f32r, tag="msg1")
                if w < GAT_BUFS:
                    nc.vector.memset(msg[:].rearrange("p c e -> p (c e)").bitcast(f32), 0.0)
                nc.gpsimd.reg_load(rlo, cnt_s[0:1, 2 * w:2 * w + 1])
                nc.gpsimd.reg_load(rhi, cnt_s[0:1, 2 * w + 1:2 * w + 2])
                nc.gpsimd.dma_gather(
                    out_ap=msg[:, 0:t_lo, :], in_ap=x_ext[:],
                    idxs_ap=ilo_s[:, w * t_lo * 8:(w + 1) * t_lo * 8],
                    num_idxs=t_lo * P, num_idxs_reg=rlo, elem_size=C_IN,
                    single_packet=False, queue_num=(w % 2) * 2,
                )
                nc.gpsimd.dma_gather(
                    out_ap=msg[:, t_lo:t_w, :], in_ap=x_ext[SPLIT:, :],
                    idxs_ap=ihi_s[:, w * t_hi * 8:(w + 1) * t_hi * 8],
                    num_idxs=t_hi * P, num_idxs_reg=rhi, elem_size=C_IN,
                    single_packet=False, queue_num=(w % 2) * 2 + 1,
                )
                agg = window_tiles(1, w, msg, C_IN)

                agg_s = wpool.tile([P, WIN], f32r, tag="agg_s")
                nc.scalar.activation(out=agg_s[:], in_=agg[:], func=COPY)
                z = ps_z.tile([P, WIN], f32, space="PSUM", tag="z")
                nc.tensor.matmul(out=z[:], lhsT=w_s[:, 0:C_HID], rhs=agg_s[:],
                                 start=True, stop=True)
                h1_s = wpool.tile([P, WIN], f32r, tag="h1")
                nc.scalar.activation(out=h1_s[:], in_=z[:], func=RELU,
                                     bias=meta_s[:, c_b1:c_b1 + 1])
                pt = ps_pt.tile([C_OUT, WIN], f32, space="PSUM", tag="pt")
                nc.tensor.matmul(out=pt[:], lhsT=w_s[:, 128:128 + C_OUT],
                                 rhs=h1_s[:], start=True, stop=True)
                pt_s = fpool.tile([C_OUT, WIN], f32r, tag="pt_s")
                nc.scalar.activation(out=pt_s[:], in_=pt[:], func=COPY)
                store_rowmajor(w, pt_s, out_ext if DEBUG_STAGE == 1 else p_mine,
                               f32 if DEBUG_STAGE == 1 else f32r)

            # ---------------- allgather ----------------
            if DEBUG_STAGE != 1:
                nc.gpsimd.collective_compute(
                    "AllGather", mybir.AluOpType.bypass,
                    replica_groups=[list(range(CORES))],
                    ins=[p_mine[:]], outs=[p_full[:]],
                )

            # ---------------- layer 2 ----------------
            for w in range(NW if DEBUG_STAGE != 1 else 0):
                msg = gpool.tile([P, t_w, C_OUT], f32r, tag="msg2")
                if w < GAT_BUFS:
                    nc.vector.memset(msg[:].rearrange("p c e -> p (c e)").bitcast(f32), 0.0)
                nc.gpsimd.reg_load(rlo, cnt_s[0:1, 2 * w:2 * w + 1])
                nc.gpsimd.reg_load(rhi, cnt_s[0:1, 2 * w + 1:2 * w + 2])
                nc.gpsimd.dma_gather(
                    out_ap=msg[:, 0:t_lo, :], in_ap=p_full[:],
                    idxs_ap=ilo_s[:, w * t_lo * 8:(w + 1) * t_lo * 8],
                    num_idxs=t_lo * P, num_idxs_reg=rlo, elem_size=C_OUT,
                    single_packet=False, queue_num=(w % 2) * 2,
                )
                nc.gpsimd.dma_gather(
                    out_ap=msg[:, t_lo:t_w, :], in_ap=p_full[SPLIT:, :],
                    idxs_ap=ihi_s[:, w * t_hi * 8:(w + 1) * t_hi * 8],
                    num_idxs=t_hi * P, num_idxs_reg=rhi, elem_size=C_OUT,
                    single_packet=False, queue_num=(w % 2) * 2 + 1,
                )
                agg = window_tiles(2, w, msg, C_OUT)

                o_s = fpool.tile([C_OUT, WIN], f32r, tag="o_s")
                nc.scalar.activation(out=o_s[:], in_=agg[:C_OUT, :], func=IDENT,
                                     bias=meta_s[0:C_OUT, c_b2:c_b2 + 1])
                store_rowmajor(w, o_s, out_ext, f32)

    _rctx.close()
    nc.compile()
    meta_layout = (c_dst, c_nrm, c_iota, c_b1, c_b2, meta_cols)
    _BUILD_CACHE[(t_lo, t_hi)] = (nc, meta_layout)
    return nc, meta_layout


def _make_inputs(x, W1, b1, W2, b2, t_lo, t_hi, per_core, meta_layout):
    c_dst, c_nrm, c_iota, c_b1, c_b2, meta_cols = meta_layout
    t_total = NW * (t_lo + t_hi)

    wts = np.zeros((P, 256), np.float32)
    wts[:, 0:128] = W1
    wts[:128, 128:192] = W2
    wts[0:64, 192:256] = np.eye(64, dtype=np.float32)

    in_maps = []
    for c in range(CORES):
        meta = np.zeros((P, meta_cols), np.float32)
        meta[:, c_dst:c_dst + t_total] = per_core[c]["dloc"]
        meta[:, c_nrm:c_nrm + t_total] = per_core[c]["nrm"]
        meta[:, c_iota:c_iota + WIN] = np.arange(WIN, dtype=np.float32)[None, :]
        meta[:, c_b1] = b1
        meta[:C_OUT, c_b2] = b2
        in_maps.append({
            "x": np.ascontiguousarray(x, dtype=np.float32),
            "idx_lo": per_core[c]["idx_lo"],
            "idx_hi": per_core[c]["idx_hi"],
            "meta": meta,
            "wts": wts,
            "cnt": per_core[c]["cnt"],
        })
    return in_maps


def kernel(x, edge_index, W1, b1, W2, b2):
    x = np.asarray(x, dtype=np.float32)
    W1 = np.asarray(W1, dtype=np.float32)
    b1 = np.asarray(b1, dtype=np.float32)
    W2 = np.asarray(W2, dtype=np.float32)
    b2 = np.asarray(b2, dtype=np.float32)

    t_lo, t_hi, per_core = _preprocess(np.asarray(edge_index))
    nc, meta_layout = _build(t_lo, t_hi)
    in_maps = _make_inputs(x, W1, b1, W2, b2, t_lo, t_hi, per_core, meta_layout)
    res = run_bass_kernel_spmd(nc, in_maps, list(range(CORES)))
    out = np.concatenate([res.results[c]["out"] for c in range(CORES)], axis=0)
    return out.astype(np.float32)

